# revision 31
# baseline (speedup 1.0000x reference)
"""Mixture-of-Experts (top-2 of 8 experts, erf-GELU FFN) on 8 Trainium2
NeuronCores, expert-parallel: core e owns expert e's weights and processes
only the tokens routed to expert e.

Host side (inside kernel()): router softmax + top-2 + renormalized combine
weights, token dispatch (gather per expert) and combine (scatter-add).
Device side (Bass/Tile SPMD): per-core FFN
    y = gelu(xg @ W1[e] + b1[e]) @ W2[e] + b2[e], scaled by combine weight,
with bf16 matmuls and fp32 accumulation.

Layouts shipped per core (P=128 partitions, C = token capacity, Cb=C/128):
  xt  [P, D/128, C]  bf16   xt[p,db,c]  = x_gathered[c, db*128+p]   (x^T)
  w1  [P, D/128, F]  bf16   w1[p,db,f]  = W1[e][db*128+p, f]
  w2  [P, F/128, D]  bf16   w2[p,fb,d]  = W2[e][fb*128+p, d]
  b1  [P, F/128]     f32    b1[p,fb]    = b1[e][fb*128+p]
  b2  [P, D]         f32    broadcast of b2[e]
  wg  [P, Cb]        f32    wg[p,cb]    = combine weight of slot cb*128+p
  out [P, Cb, D]     f32    out[p,cb,d] = y[cb*128+p, d]

MM1: hT[F,C] += W1_tile[128(D),128(F)].T @ xT[128(D), C-chunk]  (accum D)
     gelu+b1 fused in the PSUM->SBUF eviction on ScalarE (erf gelu).
MM2: y[C,D]  += hT_tile[128(F),128(C)].T @ w2[128(F), 512(D)]   (accum F)
     b2 add + combine-weight scale fused in the eviction on VectorE.
"""

import numpy as np
import ml_dtypes

P = 128
N_CORES = 8

_cache = {}
_last_in_maps = None


def _build(C, D, F):
    """Build + compile the per-core SPMD Bass program for capacity C."""
    from concourse import bacc
    import concourse.tile as tile
    import concourse.mybir as mybir

    nb_d = D // P          # D-tiles (contraction of MM1)
    nb_f = F // P          # F-tiles (contraction of MM2)
    cb_n = (C + P - 1) // P  # token tiles (last may be partial)
    nd_c = D // 512        # D output chunks of 512

    # C-chunks of <=512 for the MM1 moving operand / PSUM bank.
    # Split evenly: a tiny tail chunk (N << 128) runs LDWEIGHTS-bound on
    # the PE, so two balanced ~C/2 chunks beat (512, C-512).
    n_chunks = (C + 511) // 512
    chunks = []
    c0 = 0
    for i in range(n_chunks):
        cn = (C - c0 + (n_chunks - 1 - i)) // (n_chunks - i)
        chunks.append((c0, cn))
        c0 += cn

    bf16 = mybir.dt.bfloat16
    f32 = mybir.dt.float32
    GELU = mybir.ActivationFunctionType.Gelu

    nc = bacc.Bacc(None, target_bir_lowering=False)
    xt_d = nc.dram_tensor("xt", [P, nb_d, C], bf16, kind="ExternalInput")
    # w1 shipped tile-major [fb, p, db, q] so each per-fb load is one dense
    # per-partition-contiguous transfer (the [P, nb_d, F] layout fragments
    # into 256 B bursts and runs at ~11 GB/s)
    w1_d = nc.dram_tensor("w1", [nb_f, P, nb_d, P], bf16, kind="ExternalInput")
    w2_d = nc.dram_tensor("w2", [P, nb_f, D], bf16, kind="ExternalInput")
    b1_d = nc.dram_tensor("b1", [P, nb_f], f32, kind="ExternalInput")
    b2_d = nc.dram_tensor("b2", [P, D], f32, kind="ExternalInput")
    wg_d = nc.dram_tensor("wg", [P, cb_n], f32, kind="ExternalInput")
    out_d = nc.dram_tensor("out", [P, cb_n, D], f32, kind="ExternalOutput")

    with tile.TileContext(nc) as tc:
        with (
            tc.tile_pool(name="const", bufs=1) as const,
            tc.tile_pool(name="w1p", bufs=6) as w1p,
            tc.tile_pool(name="ps1", bufs=2, space="PSUM") as ps1p,
            tc.tile_pool(name="ps2", bufs=2, space="PSUM") as ps2p,
            tc.tile_pool(name="outp", bufs=3) as outp,
        ):
            xt_t = const.tile([P, nb_d, C], bf16)
            b1_t = const.tile([P, nb_f], f32)
            b2_t = const.tile([P, D], f32)
            wg_t = const.tile([P, cb_n], f32)
            w2_t = const.tile([P, nb_f, D], bf16)
            h_t = const.tile([P, nb_f, C], bf16)

            # critical path first: xt gates MM1 — split across the three
            # DMA-capable queues in parallel. Small constants follow on
            # GpSimd.
            splits = [(nc.sync, 0, 3), (nc.gpsimd, 3, 6), (nc.scalar, 6, nb_d)]
            for eng, a, b in splits:
                eng.dma_start(xt_t[:, a:b, :], xt_d[:, a:b, :])
            nc.gpsimd.dma_start(b1_t[:], b1_d[:])
            nc.gpsimd.dma_start(wg_t[:], wg_d[:])
            nc.gpsimd.dma_start(b2_t[:], b2_d[:])

            # ---- MM1: hT[fb] = gelu(sum_db w1[db,fb]^T @ xT[db] + b1[fb])
            # w2 (8 MiB, needed only for MM2) trickles in behind the w1
            # stream: one chunk every 4 fb so it never starves MM1's loads
            for fb in range(nb_f):
                w1_t = w1p.tile([P, nb_d, P], bf16)
                nc.sync.dma_start(w1_t[:], w1_d[fb])
                if fb % 4 == 2:
                    q, qs = fb // 4, nb_f // 8
                    nc.sync.dma_start(
                        w2_t[:, q * qs : (q + 1) * qs, :],
                        w2_d[:, q * qs : (q + 1) * qs, :],
                    )
                for c0, cn in chunks:
                    ps = ps1p.tile([P, 512], f32)
                    for db in range(nb_d):
                        nc.tensor.matmul(
                            ps[:, :cn],
                            lhsT=w1_t[:, db, :],
                            rhs=xt_t[:, db, c0 : c0 + cn],
                            start=(db == 0),
                            stop=(db == nb_d - 1),
                        )
                    nc.scalar.activation(
                        h_t[:, fb, c0 : c0 + cn],
                        ps[:, :cn],
                        GELU,
                        bias=b1_t[:, fb : fb + 1],
                    )

            # ---- MM2: y[cb, dc] = (sum_fb hT[fb,cb]^T @ w2[fb,dc]) wgt+bias
            # Last token tile may be partial (pn < 128): the matmul writes
            # only pn PSUM partitions; the eviction still reads/writes all
            # 128, but combine weights are 0 there and the host discards
            # those slots, so stale PSUM garbage is harmless.
            for cb in range(cb_n):
                pn = min(P, C - cb * P)
                o_t = outp.tile([P, D], f32)
                for dc in range(nd_c):
                    ps = ps2p.tile([P, 512], f32)
                    for fb in range(nb_f):
                        nc.tensor.matmul(
                            ps[:pn, :],
                            lhsT=h_t[:, fb, cb * P : cb * P + pn],
                            rhs=w2_t[:, fb, dc * 512 : (dc + 1) * 512],
                            start=(fb == 0),
                            stop=(fb == nb_f - 1),
                        )
                    sl = slice(dc * 512, (dc + 1) * 512)
                    nc.vector.tensor_add(o_t[:pn, sl], ps[:pn, :], b2_t[:pn, sl])
                    nc.vector.tensor_scalar_mul(
                        o_t[:pn, sl], o_t[:pn, sl], wg_t[:pn, cb : cb + 1]
                    )
                nc.sync.dma_start(out_d[:pn, cb, :], o_t[:pn, :])

    nc.compile()
    return nc


def _route(x, W_router):
    """Top-2 routing, replicating jax softmax/top_k/renorm semantics."""
    T = x.shape[0]
    logits = x @ np.asarray(W_router, np.float32)
    m = logits.max(axis=1, keepdims=True)
    ex = np.exp(logits - m, dtype=np.float32)
    probs = ex / ex.sum(axis=1, keepdims=True, dtype=np.float32)
    r = np.arange(T)
    i1 = probs.argmax(axis=1)
    masked = probs.copy()
    masked[r, i1] = -np.inf
    i2 = masked.argmax(axis=1)
    p1 = probs[r, i1]
    p2 = probs[r, i2]
    s = p1 + p2
    return i1, i2, p1 / s, p2 / s


def kernel(hidden_states, W_router, W1, b1, W2, b2):
    from concourse.bass_utils import run_bass_kernel_spmd

    B, S, D = hidden_states.shape
    E, _, F = W1.shape
    T = B * S
    x = np.ascontiguousarray(np.asarray(hidden_states, np.float32).reshape(T, D))

    i1, i2, w1c, w2c = _route(x, W_router)

    idxs, wgts = [], []
    for e in range(E):
        sel1 = i1 == e
        sel2 = i2 == e
        idx = np.nonzero(sel1 | sel2)[0]
        w = np.where(sel1[idx], w1c[idx], w2c[idx]).astype(np.float32)
        idxs.append(idx)
        wgts.append(w)

    C = max(max(len(ix) for ix in idxs), 1)
    cb_n = (C + P - 1) // P
    nb_d = D // P
    nb_f = F // P

    key = (C, D, F)
    if key not in _cache:
        _cache[key] = _build(C, D, F)
    nc = _cache[key]

    bf16 = ml_dtypes.bfloat16
    W1b = np.asarray(W1, np.float32).astype(bf16)
    W2b = np.asarray(W2, np.float32).astype(bf16)
    xb = x.astype(bf16)

    in_maps = []
    for e in range(E):
        n = len(idxs[e])
        xg = np.zeros((C, D), bf16)
        xg[:n] = xb[idxs[e]]
        xt = np.ascontiguousarray(xg.T.reshape(nb_d, P, C).transpose(1, 0, 2))
        w1e = np.ascontiguousarray(
            W1b[e].reshape(nb_d, P, nb_f, P).transpose(2, 1, 0, 3)
        )
        w2e = np.ascontiguousarray(W2b[e].reshape(nb_f, P, D).transpose(1, 0, 2))
        b1e = np.ascontiguousarray(np.asarray(b1[e], np.float32).reshape(nb_f, P).T)
        b2e = np.ascontiguousarray(
            np.broadcast_to(np.asarray(b2[e], np.float32), (P, D))
        )
        wfull = np.zeros(cb_n * P, np.float32)
        wfull[:n] = wgts[e]
        wg = np.ascontiguousarray(wfull.reshape(cb_n, P).T)
        in_maps.append(
            {"xt": xt, "w1": w1e, "w2": w2e, "b1": b1e, "b2": b2e, "wg": wg}
        )

    global _last_in_maps
    _last_in_maps = in_maps

    res = run_bass_kernel_spmd(nc, in_maps, core_ids=list(range(N_CORES)))

    out = np.zeros((T, D), np.float32)
    for e in range(E):
        n = len(idxs[e])
        y = (
            np.asarray(res.results[e]["out"])
            .transpose(1, 0, 2)
            .reshape(cb_n * P, D)[:n]
        )
        out[idxs[e]] += y
    return out.reshape(B, S, D).astype(np.float32)



# revision 39
# speedup vs baseline: 1.0582x; 1.0582x over previous
"""Mixture-of-Experts (top-2 of 8 experts, erf-GELU FFN) on 8 Trainium2
NeuronCores, expert-parallel: core e owns expert e's weights and processes
only the tokens routed to expert e.

Host side (inside kernel()): router softmax + top-2 + renormalized combine
weights, token dispatch (gather per expert) and combine (scatter-add).
Device side (Bass/Tile SPMD): per-core FFN
    y = gelu(xg @ W1[e] + b1[e]) @ W2[e] + b2[e], scaled by combine weight,
with bf16 matmuls and fp32 accumulation.

Layouts shipped per core (P=128 partitions, C = token capacity, Cb=C/128):
  xt  [P, D/128, C]  bf16   xt[p,db,c]  = x_gathered[c, db*128+p]   (x^T)
  w1  [P, D/128, F]  bf16   w1[p,db,f]  = W1[e][db*128+p, f]
  w2  [P, F/128, D]  bf16   w2[p,fb,d]  = W2[e][fb*128+p, d]
  b1  [P, F/128]     f32    b1[p,fb]    = b1[e][fb*128+p]
  b2c [P, D/128]     f32    b2c[p,dt]   = b2[e][dt*128+p]
  wgb [P, C]         f32    combine weight of slot c, broadcast over p
  out [P, D/128, C]  f32    out[p,dt,c] = y[c, dt*128+p]           (y^T)

MM1: hT[F,C] += W1_tile[128(D),128(F)].T @ xT[128(D), C-chunk]  (accum D)
     gelu+b1 fused in the PSUM->SBUF eviction on ScalarE (erf gelu).
MM2: yT[D,C] += w2_tile[128(F),128(D)].T @ hT[128(F), C-chunk]  (accum F)
     tokens stay the moving dim, so PE cost scales with the real C
     instead of padded 128-token tiles. b2 add (per-partition) +
     combine-weight multiply (free-dim row) fused in the eviction.
"""

import numpy as np
import ml_dtypes

P = 128
N_CORES = 8

_cache = {}
_last_in_maps = None


def _build(C, D, F):
    """Build + compile the per-core SPMD Bass program for capacity C."""
    from concourse import bacc
    import concourse.tile as tile
    import concourse.mybir as mybir

    nb_d = D // P          # D-tiles (contraction of MM1, output tiles of MM2)
    nb_f = F // P          # F-tiles (contraction of MM2)

    # C-chunks of <=512 for the MM1 moving operand / PSUM bank.
    # Split evenly: a tiny tail chunk (N << 128) runs LDWEIGHTS-bound on
    # the PE, so two balanced ~C/2 chunks beat (512, C-512).
    n_chunks = (C + 511) // 512
    chunks = []
    c0 = 0
    for i in range(n_chunks):
        cn = (C - c0 + (n_chunks - 1 - i)) // (n_chunks - i)
        chunks.append((c0, cn))
        c0 += cn

    bf16 = mybir.dt.bfloat16
    f32 = mybir.dt.float32
    GELU = mybir.ActivationFunctionType.Gelu

    nc = bacc.Bacc(None, target_bir_lowering=False)
    xt_d = nc.dram_tensor("xt", [P, nb_d, C], bf16, kind="ExternalInput")
    # w1 shipped tile-major [fb, p, db, q] so each per-fb load is one dense
    # per-partition-contiguous transfer (the [P, nb_d, F] layout fragments
    # into 256 B bursts and runs at ~11 GB/s)
    w1_d = nc.dram_tensor("w1", [nb_f, P, nb_d, P], bf16, kind="ExternalInput")
    w2_d = nc.dram_tensor("w2", [P, nb_f, D], bf16, kind="ExternalInput")
    b1_d = nc.dram_tensor("b1", [P, nb_f], f32, kind="ExternalInput")
    b2_d = nc.dram_tensor("b2c", [P, nb_d], f32, kind="ExternalInput")
    wg_d = nc.dram_tensor("wgb", [P, C], f32, kind="ExternalInput")
    out_d = nc.dram_tensor("out", [P, nb_d, C], f32, kind="ExternalOutput")

    with tile.TileContext(nc) as tc:
        with (
            tc.tile_pool(name="const", bufs=1) as const,
            tc.tile_pool(name="w1p", bufs=6) as w1p,
            tc.tile_pool(name="ps1", bufs=2, space="PSUM") as ps1p,
            tc.tile_pool(name="ps2", bufs=2, space="PSUM") as ps2p,
            tc.tile_pool(name="outp", bufs=3) as outp,
        ):
            xt_t = const.tile([P, nb_d, C], bf16)
            b1_t = const.tile([P, nb_f], f32)
            b2_t = const.tile([P, nb_d], f32)
            wg_t = const.tile([P, C], f32)
            w2_t = const.tile([P, nb_f, D], bf16)
            h_t = const.tile([P, nb_f, C], bf16)

            # critical path first: xt gates MM1 — split across the three
            # DMA-capable queues in parallel. Small constants follow on
            # GpSimd.
            splits = [(nc.sync, 0, 3), (nc.gpsimd, 3, 6), (nc.scalar, 6, nb_d)]
            for eng, a, b in splits:
                eng.dma_start(xt_t[:, a:b, :], xt_d[:, a:b, :])
            nc.gpsimd.dma_start(b1_t[:], b1_d[:])
            nc.gpsimd.dma_start(wg_t[:], wg_d[:])
            nc.gpsimd.dma_start(b2_t[:], b2_d[:])

            # ---- MM1: hT[fb] = gelu(sum_db w1[db,fb]^T @ xT[db] + b1[fb])
            # w2 (8 MiB, needed only for MM2) trickles in behind the w1
            # stream: one chunk every 4 fb so it never starves MM1's loads
            for fb in range(nb_f):
                w1_t = w1p.tile([P, nb_d, P], bf16)
                nc.sync.dma_start(w1_t[:], w1_d[fb])
                if fb % 4 == 2:
                    q, qs = fb // 4, nb_f // 8
                    nc.sync.dma_start(
                        w2_t[:, q * qs : (q + 1) * qs, :],
                        w2_d[:, q * qs : (q + 1) * qs, :],
                    )
                for c0, cn in chunks:
                    ps = ps1p.tile([P, 512], f32)
                    for db in range(nb_d):
                        nc.tensor.matmul(
                            ps[:, :cn],
                            lhsT=w1_t[:, db, :],
                            rhs=xt_t[:, db, c0 : c0 + cn],
                            start=(db == 0),
                            stop=(db == nb_d - 1),
                        )
                    nc.scalar.activation(
                        h_t[:, fb, c0 : c0 + cn],
                        ps[:, :cn],
                        GELU,
                        bias=b1_t[:, fb : fb + 1],
                    )

            # ---- MM2: yT[dt] = (sum_fb w2[fb,dt]^T @ hT[fb]) + b2, * wg
            # Tokens are the moving dim: PE cost scales with the real C.
            for dt in range(nb_d):
                o_t = outp.tile([P, C], f32)
                for c0, cn in chunks:
                    ps = ps2p.tile([P, 512], f32)
                    for fb in range(nb_f):
                        nc.tensor.matmul(
                            ps[:, :cn],
                            lhsT=w2_t[:, fb, dt * P : (dt + 1) * P],
                            rhs=h_t[:, fb, c0 : c0 + cn],
                            start=(fb == 0),
                            stop=(fb == nb_f - 1),
                        )
                    sl = slice(c0, c0 + cn)
                    nc.vector.tensor_scalar_add(
                        o_t[:, sl], ps[:, :cn], b2_t[:, dt : dt + 1]
                    )
                    nc.vector.tensor_mul(o_t[:, sl], o_t[:, sl], wg_t[:, sl])
                nc.sync.dma_start(out_d[:, dt, :], o_t[:])

    nc.compile()
    return nc


def _route(x, W_router):
    """Top-2 routing, replicating jax softmax/top_k/renorm semantics."""
    T = x.shape[0]
    logits = x @ np.asarray(W_router, np.float32)
    m = logits.max(axis=1, keepdims=True)
    ex = np.exp(logits - m, dtype=np.float32)
    probs = ex / ex.sum(axis=1, keepdims=True, dtype=np.float32)
    r = np.arange(T)
    i1 = probs.argmax(axis=1)
    masked = probs.copy()
    masked[r, i1] = -np.inf
    i2 = masked.argmax(axis=1)
    p1 = probs[r, i1]
    p2 = probs[r, i2]
    s = p1 + p2
    return i1, i2, p1 / s, p2 / s


def kernel(hidden_states, W_router, W1, b1, W2, b2):
    from concourse.bass_utils import run_bass_kernel_spmd

    B, S, D = hidden_states.shape
    E, _, F = W1.shape
    T = B * S
    x = np.ascontiguousarray(np.asarray(hidden_states, np.float32).reshape(T, D))

    i1, i2, w1c, w2c = _route(x, W_router)

    idxs, wgts = [], []
    for e in range(E):
        sel1 = i1 == e
        sel2 = i2 == e
        idx = np.nonzero(sel1 | sel2)[0]
        w = np.where(sel1[idx], w1c[idx], w2c[idx]).astype(np.float32)
        idxs.append(idx)
        wgts.append(w)

    C = max(max(len(ix) for ix in idxs), 1)
    nb_d = D // P
    nb_f = F // P

    key = (C, D, F)
    if key not in _cache:
        _cache[key] = _build(C, D, F)
    nc = _cache[key]

    bf16 = ml_dtypes.bfloat16
    W1b = np.asarray(W1, np.float32).astype(bf16)
    W2b = np.asarray(W2, np.float32).astype(bf16)
    xb = x.astype(bf16)

    in_maps = []
    for e in range(E):
        n = len(idxs[e])
        xg = np.zeros((C, D), bf16)
        xg[:n] = xb[idxs[e]]
        xt = np.ascontiguousarray(xg.T.reshape(nb_d, P, C).transpose(1, 0, 2))
        w1e = np.ascontiguousarray(
            W1b[e].reshape(nb_d, P, nb_f, P).transpose(2, 1, 0, 3)
        )
        w2e = np.ascontiguousarray(W2b[e].reshape(nb_f, P, D).transpose(1, 0, 2))
        b1e = np.ascontiguousarray(np.asarray(b1[e], np.float32).reshape(nb_f, P).T)
        b2e = np.ascontiguousarray(np.asarray(b2[e], np.float32).reshape(nb_d, P).T)
        wfull = np.zeros(C, np.float32)
        wfull[:n] = wgts[e]
        wgb = np.ascontiguousarray(np.broadcast_to(wfull, (P, C)))
        in_maps.append(
            {"xt": xt, "w1": w1e, "w2": w2e, "b1": b1e, "b2c": b2e, "wgb": wgb}
        )

    global _last_in_maps
    _last_in_maps = in_maps

    res = run_bass_kernel_spmd(nc, in_maps, core_ids=list(range(N_CORES)))

    out = np.zeros((T, D), np.float32)
    for e in range(E):
        n = len(idxs[e])
        # device out is y^T tiled [P, nb_d, C]: out[p, dt, c] = y[c, dt*P+p]
        y = (
            np.asarray(res.results[e]["out"])
            .transpose(2, 1, 0)
            .reshape(C, D)[:n]
        )
        out[idxs[e]] += y
    return out.reshape(B, S, D).astype(np.float32)

